# revision 1
# baseline (speedup 1.0000x reference)
"""Causal self-attention (B=2, T=2048, C=2048, H=16, D=128) on 8 trn2 cores.

Sharding: tensor-parallel over heads x data-parallel over batch.
Core c handles batch c//4, heads [4*(c%4) .. 4*(c%4)+4). Each core computes
qkv projection for its 4 heads, RoPE, causal attention, and a partial
output projection (its heads' rows of W_proj); the host sums the 4 partials
per batch.

Kernel structure (per core), all matmuls in fp32r (full PE rate @ N=512):
  Phase 1: QKV projection.
    Q,K produced transposed  (Q^T[d,t] = W_q^T x^T)  -> RoPE fused into the
    PSUM evacuation on DVE -> spilled to DRAM scratch.
    V produced natural      (V[t,d]   = x W_v)       -> DRAM scratch.
    All big DMAs split per k-tile so matmuls start as chunks land.
  Phase 2: attention per head, S^T orientation:
    S^T[k,q] = K^T.T @ Q^T   (one 128x512 matmul per tile, no contraction loop)
    causal mask on diagonal tiles = additive -1e30 on PSUM pre-exp (DVE)
    P^T = exp(S^T * 1/sqrt(D))  on ACT (no max subtraction -- scores are O(5))
    denominators: ones^T @ P^T accumulated in a [1,512] PSUM bank (PE)
    O^T[d,q] += V.T @ P^T    accumulated in PSUM over k-blocks
    normalize on evacuation: O^T * recip(broadcast(denom))
  Phase 3: partial proj: out[t,c] = sum_h O_h^T.T @ Wp_h   (O^T is already
    the required lhsT layout -- the whole kernel needs zero transposes).
"""

import contextlib
import math
import os

import numpy as np

B, T, C = 2, 2048, 2048
H, D = 16, 128
HPC = 4  # heads per core
NCORES = 8

_CACHE = {}


def _build_program():
    import concourse.tile as tile
    from concourse import bacc, mybir

    f32 = mybir.dt.float32
    f32r = mybir.dt.float32r
    Exp = mybir.ActivationFunctionType.Exp
    SCALE = 1.0 / math.sqrt(float(D))

    nc = bacc.Bacc(
        "TRN2", target_bir_lowering=False, debug=False, num_devices=NCORES
    )

    xT = nc.dram_tensor("xT", [C, T], f32r, kind="ExternalInput").ap()
    wqk = nc.dram_tensor("wqk", [C, 8 * 128], f32r, kind="ExternalInput").ap()
    wv = nc.dram_tensor("wv", [C, HPC * D], f32r, kind="ExternalInput").ap()
    wp = nc.dram_tensor("wp", [HPC * D, C], f32r, kind="ExternalInput").ap()
    onesr = nc.dram_tensor("onesr", [128, 128], f32r, kind="ExternalInput").ap()
    cosT = nc.dram_tensor("cosT", [128, T], f32, kind="ExternalInput").ap()
    sinTs = nc.dram_tensor("sinTs", [128, T], f32, kind="ExternalInput").ap()
    masks = nc.dram_tensor("masks", [4, 128, 512], f32, kind="ExternalInput").ap()
    out = nc.dram_tensor("out", [T, C], f32, kind="ExternalOutput").ap()

    KT = C // 128  # 16 contraction tiles
    NTB = T // 512  # 4 t-blocks

    with tile.TileContext(nc) as tc:
        with (
            tc.tile_pool(name="consts", bufs=1) as consts,
            tc.tile_pool(name="dram", bufs=1, space="DRAM") as dramp,
        ):
            es = contextlib.ExitStack()
            p2stp = es.enter_context(
                tc.tile_pool(name="p2st", bufs=5, space="PSUM")
            )
            cos_sb = consts.tile([128, T], f32, tag="cos")
            nc.sync.dma_start(out=cos_sb, in_=cosT)
            sin_sb = consts.tile([128, T], f32, tag="sin")
            nc.sync.dma_start(out=sin_sb, in_=sinTs)
            ones_sb = consts.tile([128, 128], f32r, tag="ones")
            nc.sync.dma_start(out=ones_sb, in_=onesr)

            # Per-chunk DRAM scratch so phase-2 loads can chase phase-1
            # writes chunk-by-chunk instead of waiting for whole tensors.
            qkt_dram = [
                [
                    dramp.tile([128, 512], f32r, tag=f"qkt{m}_{tb}",
                               name=f"qkt{m}_{tb}")
                    for tb in range(NTB)
                ]
                for m in range(8)
            ]
            vsc_dram = [
                dramp.tile([128, HPC * D], f32r, tag=f"vsc{i}", name=f"vsc{i}")
                for i in range(T // 128)
            ]

            # ---------------- Phase 1: QKV projection ----------------
            with (
                tc.tile_pool(name="p1x", bufs=2) as p1x,
                tc.tile_pool(name="p1w", bufs=1) as p1w,
                tc.tile_pool(name="p1wv", bufs=1) as p1wv,
                tc.tile_pool(name="p1e", bufs=2) as p1e,
                tc.tile_pool(name="p1ps", bufs=2, space="PSUM") as p1ps,
            ):
                # All 8 q/k weight M-tiles resident (64KB/part); chunked per k
                # and interleaved with the first x block so the first matmul
                # chain starts after ~2 chunks instead of the whole preload.
                wqkg = p1w.tile([128, KT, 8, 128], f32r, tag="wqkg")
                wv_sb = p1wv.tile([128, KT, HPC * D], f32r, tag="wv")
                xtb0 = p1x.tile([128, KT, 512], f32r, tag="xtb")
                MORD = (0, 4, 1, 5, 2, 6, 3, 7)

                def load_wm(m):
                    nc.sync.dma_start(
                        out=wqkg[:, :, m, :],
                        in_=wqk[:, m * 128 : (m + 1) * 128].rearrange(
                            "(k p) c -> p k c", p=128
                        ),
                    )

                # First compute chain (m=0) needs just its own weight column
                # and the first x chunks; stream the rest behind it.
                load_wm(MORD[0])
                load_wm(MORD[1])
                for k in range(KT):
                    nc.sync.dma_start(
                        out=xtb0[:, k], in_=xT[k * 128 : (k + 1) * 128, 0:512]
                    )
                    if k % 2 == 0 and k // 2 + 2 < 8:
                        load_wm(MORD[k // 2 + 2])
                for k in range(KT):
                    nc.sync.dma_start(
                        out=wv_sb[:, k], in_=wv[k * 128 : (k + 1) * 128, :]
                    )
                for tb in range(NTB):
                    tsl = slice(tb * 512, (tb + 1) * 512)
                    if tb == 0:
                        xtb = xtb0
                    else:
                        xtb = p1x.tile([128, KT, 512], f32r, tag="xtb",
                                       name="xtb")
                        for k in range(KT):
                            nc.sync.dma_start(
                                out=xtb[:, k],
                                in_=xT[k * 128 : (k + 1) * 128, tsl],
                            )
                    for m in (0, 4, 1, 5, 2, 6, 3, 7):
                        ps = p1ps.tile([128, 512], f32, tag="qk")
                        for k in range(KT):
                            nc.tensor.matmul(
                                ps,
                                lhsT=wqkg[:, k, m, :],
                                rhs=xtb[:, k, :],
                                start=(k == 0),
                                stop=(k == KT - 1),
                            )
                        # RoPE fused with PSUM evacuation.
                        qk_sb = p1e.tile([128, 512], f32r, tag="qke")
                        tmp = p1e.tile([128, 512], f32, tag="rtmp")
                        nc.vector.tensor_mul(
                            tmp[0:64], ps[64:128], sin_sb[0:64, tsl]
                        )
                        nc.vector.tensor_mul(
                            tmp[64:128], ps[0:64], sin_sb[64:128, tsl]
                        )
                        nc.vector.tensor_mul(qk_sb, ps, cos_sb[:, tsl])
                        nc.vector.tensor_add(qk_sb, qk_sb, tmp)
                        nc.scalar.dma_start(out=qkt_dram[m][tb], in_=qk_sb)
                    for tsub in range(4):
                        csl = slice(tsub * 128, (tsub + 1) * 128)
                        psv = p1ps.tile([128, 512], f32, tag="v", bufs=1)
                        for k in range(KT):
                            nc.tensor.matmul(
                                psv,
                                lhsT=xtb[:, k, csl],
                                rhs=wv_sb[:, k, :],
                                start=(k == 0),
                                stop=(k == KT - 1),
                            )
                        v_sb = p1e.tile([128, 512], f32r, tag="ve")
                        nc.scalar.copy(v_sb, psv)
                        nc.scalar.dma_start(
                            out=vsc_dram[tb * 4 + tsub], in_=v_sb
                        )

            # ---------------- Phases 2+3 share the O^T tiles -------------
            with tc.tile_pool(name="o2", bufs=1) as o2p:
                out2T = [
                    o2p.tile([128, T], f32r, tag=f"o2_{h}", name=f"o2_{h}")
                    for h in range(HPC)
                ]
                _phase2(tc, nc, f32, f32r, Exp, SCALE, KT, qkt_dram,
                        vsc_dram, masks, ones_sb, out2T, p2stp)
                es.close()
                _phase3(tc, nc, f32, f32r, out2T, wp, out)
    nc.compile()
    return nc


def _phase2(tc, nc, f32, f32r, Exp, SCALE, KT, qkt_dram, vsc_dram, masks,
            ones_sb, out2T, p2stp):
    with (
        tc.tile_pool(name="p2m", bufs=1) as p2m,
        tc.tile_pool(name="p2qkv", bufs=2) as p2qkv,
        tc.tile_pool(name="p2pt", bufs=3) as p2pt,
        tc.tile_pool(name="p2s", bufs=2) as p2s,
        tc.tile_pool(name="p2pv", bufs=2, space="PSUM") as p2pv,
        tc.tile_pool(name="p2dn", bufs=1, space="PSUM") as p2dn,
    ):
        mask_sb = p2m.tile([128, 4, 512], f32, tag="mask")
        nc.sync.dma_start(out=mask_sb, in_=masks.rearrange("j p q -> p j q"))
        for h in range(HPC):
            qt = p2qkv.tile([128, T], f32r, tag="qt")
            kt = p2qkv.tile([128, T], f32r, tag="kt")
            vt = p2qkv.tile([128, KT, 128], f32r, tag="vt")
            for tb in range(4):
                s = slice(tb * 512, (tb + 1) * 512)
                nc.sync.dma_start(out=kt[:, s], in_=qkt_dram[4 + h][tb])
                nc.sync.dma_start(out=qt[:, s], in_=qkt_dram[h][tb])
                for tsub in range(4):
                    i = tb * 4 + tsub
                    nc.sync.dma_start(
                        out=vt[:, i],
                        in_=vsc_dram[i][:, h * 128 : (h + 1) * 128],
                    )
            for qb in range(4):  # ascending: chases phase-1 output chunks
                qsl = slice(qb * 512, (qb + 1) * 512)
                pv = p2pv.tile([128, 512], f32, tag="pv")
                dn = p2dn.tile([128, 512], f32, tag="dn")
                nk = 4 * (qb + 1)
                for kb in range(nk):
                    st = p2stp.tile([128, 512], f32, tag="st")
                    nc.tensor.matmul(
                        st,
                        lhsT=kt[:, kb * 128 : (kb + 1) * 128],
                        rhs=qt[:, qsl],
                        start=True,
                        stop=True,
                    )
                    if kb >= qb * 4:
                        nc.vector.tensor_add(st, st, mask_sb[:, kb - qb * 4, :])
                    pt = p2pt.tile([128, 512], f32r, tag="pt")
                    nc.scalar.activation(pt, st, Exp, scale=SCALE)
                    nc.tensor.matmul(
                        dn,
                        lhsT=ones_sb,
                        rhs=pt,
                        start=(kb == 0),
                        stop=(kb == nk - 1),
                    )
                    nc.tensor.matmul(
                        pv,
                        lhsT=vt[:, kb, :],
                        rhs=pt,
                        start=(kb == 0),
                        stop=(kb == nk - 1),
                    )
                # dn already holds the denominator on every partition
                # (ones[128,128] lhsT): reciprocal + normalize, no broadcast.
                rb2 = p2s.tile([128, 512], f32, tag="rb2")
                nc.vector.reciprocal_approx_fast(out=rb2, in_=dn)
                nc.vector.tensor_mul(out2T[h][:, qsl], pv, rb2)


def _phase3(tc, nc, f32, f32r, out2T, wp, out):
    with (
        tc.tile_pool(name="p3w", bufs=1) as p3w,
        tc.tile_pool(name="p3e", bufs=4) as p3e,
        tc.tile_pool(name="p3ps", bufs=8, space="PSUM") as p3ps,
    ):
        wps = [
            p3w.tile([128, T], f32r, tag=f"wp{i}", name=f"wp{i}")
            for i in range(HPC)
        ]
        for i in range(HPC):
            nc.sync.dma_start(out=wps[i], in_=wp[i * 128 : (i + 1) * 128, :])
        for t in range(T // 128):
            tsl = slice(t * 128, (t + 1) * 128)
            pos = [
                p3ps.tile([128, 512], f32, tag="po", name=f"po{t}_{cb}")
                for cb in range(4)
            ]
            # hd outer / cb inner: 4 matmuls share one LDWEIGHTS.
            for hd in range(HPC):
                for cb in range(4):
                    nc.tensor.matmul(
                        pos[cb],
                        lhsT=out2T[hd][:, tsl],
                        rhs=wps[hd][:, cb * 512 : (cb + 1) * 512],
                        start=(hd == 0),
                        stop=(hd == HPC - 1),
                    )
            for cb in range(4):
                ob = p3e.tile([128, 512], f32, tag="ob")
                nc.vector.tensor_copy(ob, pos[cb])
                nc.sync.dma_start(
                    out=out[tsl, cb * 512 : (cb + 1) * 512], in_=ob
                )


def _get_program():
    if "nc" not in _CACHE:
        _CACHE["nc"] = _build_program()
    return _CACHE["nc"]


def make_in_maps(x, cos, sin, W_qkv, W_proj):
    """Host-side sharding: per-core input dicts (numpy, fp32)."""
    x = np.asarray(x, dtype=np.float32)
    cos = np.asarray(cos, dtype=np.float32)
    sin = np.asarray(sin, dtype=np.float32)
    W_qkv = np.asarray(W_qkv, dtype=np.float32)
    W_proj = np.asarray(W_proj, dtype=np.float32)

    cosT = np.ascontiguousarray(np.tile(cos.T, (2, 1)))  # [128, T]
    sinT = np.ascontiguousarray(np.concatenate([-sin.T, sin.T], axis=0))
    q_idx = np.arange(512)[None, None, :]
    k_idx = np.arange(128)[None, :, None]
    j_idx = np.arange(4)[:, None, None]
    masks = np.where(
        q_idx >= j_idx * 128 + k_idx, 0.0, -1.0e30
    ).astype(np.float32)  # [4, 128, 512] additive
    onesr = np.ones((128, 128), dtype=np.float32)

    in_maps = []
    for core in range(NCORES):
        b, hg = core // 4, core % 4
        csl = slice(hg * 512, (hg + 1) * 512)
        wqk_np = np.ascontiguousarray(
            np.concatenate(
                [W_qkv[:, csl], W_qkv[:, C + hg * 512 : C + (hg + 1) * 512]],
                axis=1,
            )
        )
        wv_np = np.ascontiguousarray(
            W_qkv[:, 2 * C + hg * 512 : 2 * C + (hg + 1) * 512]
        )
        wp_np = np.ascontiguousarray(W_proj[hg * 512 : (hg + 1) * 512, :])
        xT_np = np.ascontiguousarray(x[b].T)
        in_maps.append(
            {
                "xT": xT_np,
                "wqk": wqk_np,
                "wv": wv_np,
                "wp": wp_np,
                "onesr": onesr,
                "cosT": cosT,
                "sinTs": sinT,
                "masks": masks,
            }
        )
    return in_maps


def kernel(x, cos, sin, W_qkv, W_proj):
    from concourse.bass_utils import run_bass_kernel_spmd

    nc = _get_program()
    in_maps = make_in_maps(x, cos, sin, W_qkv, W_proj)
    trace = bool(int(os.environ.get("KERNEL_TRACE", "0")))
    res = run_bass_kernel_spmd(
        nc, in_maps, core_ids=list(range(NCORES)), trace=trace
    )
    if trace:
        _CACHE["last_results"] = res
        if res.exec_time_ns is not None:
            print(f"HW exec time: {res.exec_time_ns} ns")

    out = np.zeros((B, T, C), dtype=np.float32)
    for core in range(NCORES):
        out[core // 4] += res.results[core]["out"]
    return out



# revision 3
# speedup vs baseline: 1.3421x; 1.3421x over previous
"""Causal self-attention (B=2, T=2048, C=2048, H=16, D=128) on 8 trn2 cores.

Sharding: tensor-parallel over heads x data-parallel over batch.
Core c handles batch c//4, heads [4*(c%4) .. 4*(c%4)+4). Each core computes
qkv projection for its 4 heads, RoPE, causal attention, and a partial
output projection (its heads' rows of W_proj); the host sums the 4 partials
per batch.

All matmul operands are bf16 (PSUM accumulation stays fp32): bf16 enables
the fast-weight-load path so LDWEIGHTS fully overlaps the previous matmul
(fp32 HIGH mode serialized a 128-cycle LDW per matmul), and it halves every
DMA byte. Q^T/K^T/V stay resident in SBUF between phases (no DRAM scratch).

Kernel structure (per core):
  Phase 1: QKV projection from x^T (bf16).
    Q,K produced transposed (Q^T[d,t] = W_q^T x^T) -> RoPE fused into the
    PSUM evacuation on DVE, written bf16 straight into resident SBUF tiles.
    V produced natural (V[t,d] = x W_v) -> resident SBUF tile via ACT copy.
  Phase 2+3 fused, qb-outer / head-inner, software-pipelined one S^T tile
  ahead so the PE never waits on the ACT exp:
    S^T[k,q] = K^T.T @ Q^T   (diagonal tiles trimmed to their exact causal
    width; the strictly-upper 128x128 triangle gets an additive -1e30 mask)
    P^T = exp(S^T * 1/sqrt(D)) on ACT (no max subtraction; scores are O(5))
    denominator: P^T tiles accumulated into an SBUF f32 tile on DVE, then a
    single ones^T @ acc matmul per (qb, head) replicates the per-q sums
    across partitions (vs. one matmul per k-tile = 33% more PE work).
    O^T[d,q] += V.T @ P^T accumulated in PSUM over k-tiles, normalized on
    evacuation by reciprocal(denominator).
    After all 4 heads finish a qb block, its rows of the output projection
    run immediately (out[t,c] = sum_h O_h^T.T @ Wp_h) and stream to DRAM,
    keeping the PE dense and letting ACT run ahead.
"""

import math
import os

import numpy as np

B, T, C = 2, 2048, 2048
H, D = 16, 128
HPC = 4  # heads per core
NCORES = 8
KT = C // 128  # 16 contraction tiles
NTB = T // 512  # 4 t-blocks

_CACHE = {}


def _build_program():
    import concourse.tile as tile
    from concourse import bacc, mybir

    f32 = mybir.dt.float32
    f32r = mybir.dt.float32r
    bf16 = mybir.dt.bfloat16
    Exp = mybir.ActivationFunctionType.Exp
    SCALE = 1.0 / math.sqrt(float(D))

    nc = bacc.Bacc(
        "TRN2", target_bir_lowering=False, debug=False, num_devices=NCORES
    )

    xT = nc.dram_tensor("xT", [C, T], bf16, kind="ExternalInput").ap()
    wqkg = nc.dram_tensor(
        "wqkg", [8, 128, KT * 128], bf16, kind="ExternalInput"
    ).ap()
    wv = nc.dram_tensor("wv", [128, KT * 512], bf16, kind="ExternalInput").ap()
    wp = nc.dram_tensor("wp", [HPC * 128, C], bf16, kind="ExternalInput").ap()
    onesr = nc.dram_tensor("onesr", [128, 128], f32r, kind="ExternalInput").ap()
    cosT = nc.dram_tensor("cosT", [128, T], f32, kind="ExternalInput").ap()
    sinTs = nc.dram_tensor("sinTs", [128, T], f32, kind="ExternalInput").ap()
    trimask = nc.dram_tensor(
        "trimask", [128, 128], f32, kind="ExternalInput"
    ).ap()
    out = nc.dram_tensor("out", [T, C], f32, kind="ExternalOutput").ap()

    with tile.TileContext(nc) as tc:
        with (
            tc.tile_pool(name="consts", bufs=1) as consts,
            tc.tile_pool(name="persist", bufs=1) as pers,
        ):
            ones_sb = consts.tile([128, 128], f32r, tag="ones")
            nc.sync.dma_start(out=ones_sb, in_=onesr)
            mask_sb = consts.tile([128, 128], f32, tag="mask")
            nc.sync.dma_start(out=mask_sb, in_=trimask)

            qts = [
                pers.tile([128, T], bf16, tag=f"qt{h}", name=f"qt{h}")
                for h in range(HPC)
            ]
            kts = [
                pers.tile([128, T], bf16, tag=f"kt{h}", name=f"kt{h}")
                for h in range(HPC)
            ]
            vt = pers.tile([128, KT, 512], bf16, tag="vt", name="vt")
            o2 = [
                pers.tile([128, T], bf16, tag=f"o2{h}", name=f"o2{h}")
                for h in range(HPC)
            ]

            # ---------------- Phase 1: QKV projection ----------------
            with (
                tc.tile_pool(name="p1w", bufs=1) as p1w,
                tc.tile_pool(name="p1x", bufs=2) as p1x,
                tc.tile_pool(name="p1e", bufs=3) as p1e,
                tc.tile_pool(name="p1ps", bufs=2, space="PSUM") as p1ps,
                tc.tile_pool(name="p1pv", bufs=2, space="PSUM") as p1pv,
            ):
                cos_sb = p1w.tile([128, T], f32, tag="cos")
                nc.sync.dma_start(out=cos_sb, in_=cosT)
                sin_sb = p1w.tile([128, T], f32, tag="sin")
                nc.sync.dma_start(out=sin_sb, in_=sinTs)

                # m 0-3: W_q columns per head, 4-7: W_k. Host pre-packs each
                # m's [128, KT*128] lhsT block contiguously.
                wqkg_sb = p1w.tile([128, 8, KT * 128], bf16, tag="wqkg")
                wv_sb = p1w.tile([128, KT, 512], bf16, tag="wv")
                MORD = (0, 4, 1, 5, 2, 6, 3, 7)

                def load_wm(m):
                    nc.sync.dma_start(out=wqkg_sb[:, m, :], in_=wqkg[m])

                # First compute chain (m=0) needs just its own weight block
                # and the first x chunks; stream the rest behind it.
                load_wm(MORD[0])
                load_wm(MORD[1])
                xtb0 = p1x.tile([128, KT, 512], bf16, tag="xtb", name="xtb0")
                for k in range(KT):
                    nc.sync.dma_start(
                        out=xtb0[:, k], in_=xT[k * 128 : (k + 1) * 128, 0:512]
                    )
                    if k % 2 == 0 and k // 2 + 2 < 8:
                        load_wm(MORD[k // 2 + 2])
                nc.sync.dma_start(
                    out=wv_sb, in_=wv.rearrange("p (k c) -> p k c", k=KT)
                )

                for tb in range(NTB):
                    tsl = slice(tb * 512, (tb + 1) * 512)
                    if tb == 0:
                        xtb = xtb0
                    else:
                        xtb = p1x.tile(
                            [128, KT, 512], bf16, tag="xtb", name=f"xtb{tb}"
                        )
                        for k in range(KT):
                            nc.sync.dma_start(
                                out=xtb[:, k],
                                in_=xT[k * 128 : (k + 1) * 128, tsl],
                            )
                    for m in MORD:
                        ps = p1ps.tile([128, 512], f32, tag="qk")
                        for k in range(KT):
                            nc.tensor.matmul(
                                ps,
                                lhsT=wqkg_sb[:, m, k * 128 : (k + 1) * 128],
                                rhs=xtb[:, k, :],
                                start=(k == 0),
                                stop=(k == KT - 1),
                            )
                        # RoPE fused with PSUM evacuation, bf16 out.
                        dst = (qts[m] if m < 4 else kts[m - 4])[:, tsl]
                        tmp = p1e.tile([128, 512], f32, tag="rtmp")
                        nc.vector.tensor_mul(
                            tmp[0:64], ps[64:128], sin_sb[0:64, tsl]
                        )
                        nc.vector.tensor_mul(
                            tmp[64:128], ps[0:64], sin_sb[64:128, tsl]
                        )
                        nc.vector.tensor_mul(dst, ps, cos_sb[:, tsl])
                        nc.vector.tensor_add(dst, dst, tmp)
                    for tsub in range(4):
                        csl = slice(tsub * 128, (tsub + 1) * 128)
                        psv = p1pv.tile([128, 512], f32, tag="v")
                        for k in range(KT):
                            nc.tensor.matmul(
                                psv,
                                lhsT=xtb[:, k, csl],
                                rhs=wv_sb[:, k],
                                start=(k == 0),
                                stop=(k == KT - 1),
                            )
                        nc.scalar.copy(vt[:, tb * 4 + tsub, :], psv)

            # ---------------- Phases 2+3 fused ----------------
            with (
                tc.tile_pool(name="p3w", bufs=1) as p3w,
                tc.tile_pool(name="p2pt", bufs=3) as p2pt,
                tc.tile_pool(name="p2acc", bufs=2) as p2acc,
                tc.tile_pool(name="p2s", bufs=2) as p2s,
                tc.tile_pool(name="p3e", bufs=4) as p3e,
                tc.tile_pool(name="p2st", bufs=3, space="PSUM") as p2st,
                tc.tile_pool(name="p2pv", bufs=2, space="PSUM") as p2pv,
                tc.tile_pool(name="p2dn", bufs=1, space="PSUM") as p2dn,
                tc.tile_pool(name="p3ps", bufs=2, space="PSUM") as p3ps,
            ):
                wps = [
                    p3w.tile([128, T], bf16, tag=f"wp{i}", name=f"wp{i}")
                    for i in range(HPC)
                ]
                for i in range(HPC):
                    nc.sync.dma_start(
                        out=wps[i], in_=wp[i * 128 : (i + 1) * 128, :]
                    )

                # Flat tile list: (qb, h, kb, o, w). Diagonal tiles (kb in
                # [4qb, 4qb+4)) are trimmed to their causal width; the first
                # 128 cols of a diagonal tile get the triangular mask.
                tiles = []
                for qb in range(4):
                    for h in range(HPC):
                        nk = 4 * (qb + 1)
                        for kb in range(nk):
                            j = kb - qb * 4
                            o = j * 128 if j >= 0 else 0
                            tiles.append((qb, h, kb, o, 512 - o, j >= 0))

                def emit_st(t):
                    qb, h, kb, o, w, diag = t
                    st = p2st.tile([128, 512], f32, tag="st")
                    nc.tensor.matmul(
                        st[:, 0:w],
                        lhsT=kts[h][:, kb * 128 : (kb + 1) * 128],
                        rhs=qts[h][:, qb * 512 + o : (qb + 1) * 512],
                        start=True,
                        stop=True,
                    )
                    return st

                st_next = emit_st(tiles[0])
                cur = None
                for i, t in enumerate(tiles):
                    qb, h, kb, o, w, diag = t
                    st = st_next
                    if i + 1 < len(tiles):
                        st_next = emit_st(tiles[i + 1])
                    if diag:
                        nc.vector.tensor_add(st[:, 0:128], st[:, 0:128], mask_sb)
                    pt = p2pt.tile([128, 512], bf16, tag="pt")
                    nc.scalar.activation(pt[:, 0:w], st[:, 0:w], Exp, scale=SCALE)
                    nk = 4 * (qb + 1)
                    if kb == 0:
                        acc_t = p2acc.tile(
                            [128, 512], f32r, tag="acc", name="acc"
                        )
                        pv_t = p2pv.tile([128, 512], f32, tag="pv", name="pv")
                        cur = (acc_t, pv_t)
                    acc, pv = cur
                    if kb == 0:
                        nc.vector.tensor_copy(acc, pt)
                    else:
                        nc.vector.tensor_add(
                            acc[:, o:512], acc[:, o:512], pt[:, 0:w]
                        )
                    nc.tensor.matmul(
                        pv[:, o:512],
                        lhsT=vt[:, kb, h * 128 : (h + 1) * 128],
                        rhs=pt[:, 0:w],
                        start=(kb == 0),
                        stop=(kb == nk - 1),
                    )
                    if kb == nk - 1:
                        # Denominator: one matmul replicates per-q column
                        # sums of acc across all partitions.
                        dn = p2dn.tile([128, 512], f32, tag="dn")
                        nc.tensor.matmul(
                            dn, lhsT=ones_sb, rhs=acc, start=True, stop=True
                        )
                        rb = p2s.tile([128, 512], f32, tag="rb")
                        nc.vector.reciprocal_approx_fast(out=rb, in_=dn)
                        qsl = slice(qb * 512, (qb + 1) * 512)
                        nc.vector.tensor_mul(o2[h][:, qsl], pv, rb)
                        if h == HPC - 1:
                            # Phase 3 for this qb's 4 row-tiles.
                            for ts2 in range(4):
                                t0 = qb * 512 + ts2 * 128
                                trow = slice(t0, t0 + 128)
                                for cb in range(4):
                                    pos = p3ps.tile([128, 512], f32, tag="pos")
                                    for hd in range(HPC):
                                        nc.tensor.matmul(
                                            pos,
                                            lhsT=o2[hd][:, trow],
                                            rhs=wps[hd][
                                                :, cb * 512 : (cb + 1) * 512
                                            ],
                                            start=(hd == 0),
                                            stop=(hd == HPC - 1),
                                        )
                                    ob = p3e.tile([128, 512], f32, tag="ob")
                                    nc.vector.tensor_copy(ob, pos)
                                    nc.sync.dma_start(
                                        out=out[trow, cb * 512 : (cb + 1) * 512],
                                        in_=ob,
                                    )
    nc.compile()
    return nc


def _get_program():
    if "nc" not in _CACHE:
        _CACHE["nc"] = _build_program()
    return _CACHE["nc"]


def make_in_maps(x, cos, sin, W_qkv, W_proj):
    """Host-side sharding: per-core input dicts."""
    import ml_dtypes

    bf16 = ml_dtypes.bfloat16
    x = np.asarray(x, dtype=np.float32)
    cos = np.asarray(cos, dtype=np.float32)
    sin = np.asarray(sin, dtype=np.float32)
    W_qkv = np.asarray(W_qkv, dtype=np.float32)
    W_proj = np.asarray(W_proj, dtype=np.float32)

    cosT = np.ascontiguousarray(np.tile(cos.T, (2, 1)))  # [128, T]
    sinT = np.ascontiguousarray(np.concatenate([-sin.T, sin.T], axis=0))
    k_idx = np.arange(128)[:, None]
    c_idx = np.arange(128)[None, :]
    trimask = np.where(k_idx <= c_idx, 0.0, -1.0e30).astype(np.float32)
    onesr = np.ones((128, 128), dtype=np.float32)

    in_maps = []
    for core in range(NCORES):
        b, hg = core // 4, core % 4
        csl = slice(hg * 512, (hg + 1) * 512)
        wqk_np = np.concatenate(
            [W_qkv[:, csl], W_qkv[:, C + hg * 512 : C + (hg + 1) * 512]],
            axis=1,
        )  # [C, 1024]
        # lhsT blocks per m-tile, contiguous: [8, 128, KT*128]
        wqkg_np = np.ascontiguousarray(
            wqk_np.reshape(KT, 128, 8, 128)
            .transpose(2, 1, 0, 3)
            .reshape(8, 128, KT * 128)
            .astype(bf16)
        )
        wv_np = np.ascontiguousarray(
            W_qkv[:, 2 * C + hg * 512 : 2 * C + (hg + 1) * 512]
            .reshape(KT, 128, 512)
            .transpose(1, 0, 2)
            .reshape(128, KT * 512)
            .astype(bf16)
        )
        wp_np = np.ascontiguousarray(
            W_proj[hg * 512 : (hg + 1) * 512, :].astype(bf16)
        )
        xT_np = np.ascontiguousarray(x[b].T.astype(bf16))
        in_maps.append(
            {
                "xT": xT_np,
                "wqkg": wqkg_np,
                "wv": wv_np,
                "wp": wp_np,
                "onesr": onesr,
                "cosT": cosT,
                "sinTs": sinT,
                "trimask": trimask,
            }
        )
    return in_maps


def kernel(x, cos, sin, W_qkv, W_proj):
    from concourse.bass_utils import run_bass_kernel_spmd

    nc = _get_program()
    in_maps = make_in_maps(x, cos, sin, W_qkv, W_proj)
    trace = bool(int(os.environ.get("KERNEL_TRACE", "0")))
    res = run_bass_kernel_spmd(
        nc, in_maps, core_ids=list(range(NCORES)), trace=trace
    )
    if trace:
        _CACHE["last_results"] = res
        if res.exec_time_ns is not None:
            print(f"HW exec time: {res.exec_time_ns} ns")

    out = np.zeros((B, T, C), dtype=np.float32)
    for core in range(NCORES):
        out[core // 4] += res.results[core]["out"]
    return out


# revision 4
# speedup vs baseline: 1.4685x; 1.0942x over previous
"""Causal self-attention (B=2, T=2048, C=2048, H=16, D=128) on 8 trn2 cores.

Sharding: tensor-parallel over heads x data-parallel over batch.
Core c handles batch c//4, heads [4*(c%4) .. 4*(c%4)+4). Each core computes
qkv projection for its 4 heads, RoPE, causal attention, and a partial
output projection (its heads' rows of W_proj); the host sums the 4 partials
per batch.

All matmul operands are bf16 (PSUM accumulation stays fp32): bf16 enables
the fast-weight-load path so LDWEIGHTS fully overlaps the previous matmul
(fp32 HIGH mode serializes a 128-cycle LDW per matmul). Q^T/K^T/V stay
resident in SBUF (no DRAM scratch).

The whole kernel is one software-pipelined stream, interleaved so the
ACT (exp) and DVE (RoPE/mask/denominator-accumulate/evacuations) work of
attention hides under PE-dense projection chains:

  seg 0: qkv chains t-block 0
  seg 1: qkv chains tb1  + attention q-block 0 tiles as fillers
  seg 2: qkv chains tb2  + attention qb1 + out-proj rows of qb0
  seg 3: qkv chains tb3  + attention qb2 + out-proj qb1
  seg 4: attention qb3 with out-proj qb2 as filler
  seg 5: out-proj qb3

Attention (S^T orientation, one tile in flight ahead of the exp):
  S^T[k,q] = K^T.T @ Q^T   (diagonal tiles trimmed to exact causal width;
  the strictly-upper 128x128 triangle gets an additive -1e30 mask on DVE)
  P^T = exp(S^T / sqrt(D)) on ACT, bf16 (no max subtraction: scores O(5))
  denominator: P^T tiles accumulated into an SBUF f32 tile on DVE; one
  ones^T @ acc matmul per (qb, head) replicates per-q sums across
  partitions (saves the per-tile ones-matmul's 33% extra PE streaming)
  O^T[d,q] += V.T @ P^T in PSUM, normalized by reciprocal(denom) on DVE.
Out-projection (out[t,c] = sum_h O_h^T.T @ Wp_h) consumes O^T directly;
evacuations alternate DVE/ACT to balance engine load.
"""

import math
import os

import numpy as np

B, T, C = 2, 2048, 2048
H, D = 16, 128
HPC = 4  # heads per core
NCORES = 8
KT = C // 128  # 16 contraction tiles
NTB = T // 512  # 4 t-blocks

_CACHE = {}


def _build_program():
    import concourse.tile as tile
    from concourse import bacc, mybir

    f32 = mybir.dt.float32
    f32r = mybir.dt.float32r
    bf16 = mybir.dt.bfloat16
    Exp = mybir.ActivationFunctionType.Exp
    SCALE = 1.0 / math.sqrt(float(D))

    nc = bacc.Bacc(
        "TRN2", target_bir_lowering=False, debug=False, num_devices=NCORES
    )

    xT = nc.dram_tensor("xT", [C, T], bf16, kind="ExternalInput").ap()
    wqkg = nc.dram_tensor(
        "wqkg", [8, 128, KT * 128], bf16, kind="ExternalInput"
    ).ap()
    wv = nc.dram_tensor("wv", [128, KT * 512], bf16, kind="ExternalInput").ap()
    wp = nc.dram_tensor("wp", [HPC * 128, C], bf16, kind="ExternalInput").ap()
    onesr = nc.dram_tensor("onesr", [128, 128], f32r, kind="ExternalInput").ap()
    cosT = nc.dram_tensor("cosT", [128, T], bf16, kind="ExternalInput").ap()
    sinTs = nc.dram_tensor("sinTs", [128, T], bf16, kind="ExternalInput").ap()
    trimask = nc.dram_tensor(
        "trimask", [128, 128], f32, kind="ExternalInput"
    ).ap()
    out = nc.dram_tensor("out", [T, C], f32, kind="ExternalOutput").ap()

    with tile.TileContext(nc) as tc:
        with (
            tc.tile_pool(name="consts", bufs=1) as consts,
            tc.tile_pool(name="pers", bufs=1) as pers,
            tc.tile_pool(name="ppt", bufs=3) as ppt,
            tc.tile_pool(name="pacc", bufs=2) as pacc,
            tc.tile_pool(name="prb", bufs=2) as prb,
            tc.tile_pool(name="pob", bufs=4) as pob,
            tc.tile_pool(name="p1w", bufs=1) as p1w,
            tc.tile_pool(name="p1x", bufs=2) as p1x,
            tc.tile_pool(name="p1e", bufs=3) as p1e,
            tc.tile_pool(name="psA", bufs=2, space="PSUM") as psA,
            tc.tile_pool(name="psST", bufs=2, space="PSUM") as psST,
            tc.tile_pool(name="psPV", bufs=2, space="PSUM") as psPV,
            tc.tile_pool(name="psX", bufs=2, space="PSUM") as psX,
        ):
            # ---- persistent SBUF tensors ----
            qts = [
                pers.tile([128, T], bf16, tag=f"qt{h}", name=f"qt{h}")
                for h in range(HPC)
            ]
            kts = [
                pers.tile([128, T], bf16, tag=f"kt{h}", name=f"kt{h}")
                for h in range(HPC)
            ]
            vt = pers.tile([128, KT, 512], bf16, tag="vt", name="vt")
            o2 = [
                pers.tile([128, T], bf16, tag=f"o2{h}", name=f"o2{h}")
                for h in range(HPC)
            ]
            wps = [
                pers.tile([128, T], bf16, tag=f"wp{i}", name=f"wp{i}")
                for i in range(HPC)
            ]

            # ---- input DMAs: weights + first x block first, the rest
            # behind them; big/cold loads on the scalar queue. ----
            wqkg_sb = p1w.tile([128, 8, KT * 128], bf16, tag="wqkg")
            wv_sb = p1w.tile([128, KT, 512], bf16, tag="wv")
            cos_sb = p1w.tile([128, T], bf16, tag="cos")
            sin_sb = p1w.tile([128, T], bf16, tag="sin")
            ones_sb = consts.tile([128, 128], f32r, tag="ones")
            mask_sb = consts.tile([128, 128], f32, tag="mask")

            MORD = (0, 4, 1, 5, 2, 6, 3, 7)
            xtbs = [None] * NTB

            def load_x(tb):
                def go():
                    xtb = p1x.tile(
                        [128, KT, 512], bf16, tag="xtb", name=f"xtb{tb}"
                    )
                    xtbs[tb] = xtb
                    tsl = slice(tb * 512, (tb + 1) * 512)
                    for k in range(KT):
                        nc.sync.dma_start(
                            out=xtb[:, k], in_=xT[k * 128 : (k + 1) * 128, tsl]
                        )
                return go

            def load_wm(m):
                nc.sync.dma_start(out=wqkg_sb[:, m, :], in_=wqkg[m])

            load_wm(MORD[0])
            load_x(0)()
            load_wm(MORD[1])
            for mi in range(2, 8):
                load_wm(MORD[mi])
            nc.scalar.dma_start(out=ones_sb, in_=onesr)
            nc.scalar.dma_start(out=mask_sb, in_=trimask)
            nc.scalar.dma_start(out=cos_sb, in_=cosT)
            nc.scalar.dma_start(out=sin_sb, in_=sinTs)
            nc.scalar.dma_start(
                out=wv_sb, in_=wv.rearrange("p (k c) -> p k c", k=KT)
            )
            for i in range(HPC):
                nc.scalar.dma_start(
                    out=wps[i], in_=wp[i * 128 : (i + 1) * 128, :]
                )

            # ---- phase-1 chain closures ----
            def chain_qk(tb, m):
                def go():
                    tsl = slice(tb * 512, (tb + 1) * 512)
                    ps = psA.tile([128, 512], f32, tag="chain", name="psqk")
                    for k in range(KT):
                        nc.tensor.matmul(
                            ps,
                            lhsT=wqkg_sb[:, m, k * 128 : (k + 1) * 128],
                            rhs=xtbs[tb][:, k, :],
                            start=(k == 0),
                            stop=(k == KT - 1),
                        )
                    # RoPE fused with PSUM evacuation, bf16 out.
                    dst = (qts[m] if m < 4 else kts[m - 4])[:, tsl]
                    tmp = p1e.tile([128, 512], f32, tag="rtmp", name="rtmp")
                    nc.vector.tensor_mul(
                        tmp[0:64], ps[64:128], sin_sb[0:64, tsl]
                    )
                    nc.vector.tensor_mul(
                        tmp[64:128], ps[0:64], sin_sb[64:128, tsl]
                    )
                    nc.vector.tensor_mul(dst, ps, cos_sb[:, tsl])
                    nc.vector.tensor_add(dst, dst, tmp)
                return go

            def chain_v(tb, tsub):
                def go():
                    csl = slice(tsub * 128, (tsub + 1) * 128)
                    psv = psA.tile([128, 512], f32, tag="chain", name="psv")
                    for k in range(KT):
                        nc.tensor.matmul(
                            psv,
                            lhsT=xtbs[tb][:, k, csl],
                            rhs=wv_sb[:, k],
                            start=(k == 0),
                            stop=(k == KT - 1),
                        )
                    nc.scalar.copy(vt[:, tb * 4 + tsub, :], psv)
                return go

            def chains(tb):
                cs = [chain_qk(tb, m) for m in MORD]
                cs += [chain_v(tb, tsub) for tsub in range(4)]
                return cs

            # ---- attention step closures for one q-block ----
            def att_steps(qb):
                nk = 4 * (qb + 1)
                tiles = []
                for h in range(HPC):
                    for kb in range(nk):
                        j = kb - qb * 4
                        o = j * 128 if j >= 0 else 0
                        tiles.append((h, kb, o, 512 - o, j >= 0))
                state = {}
                grp = {}

                def emit_st(i):
                    h, kb, o, w, diag = tiles[i]
                    st = psST.tile([128, 512], f32, tag="st", name="st")
                    nc.tensor.matmul(
                        st[:, 0:w],
                        lhsT=kts[h][:, kb * 128 : (kb + 1) * 128],
                        rhs=qts[h][:, qb * 512 + o : (qb + 1) * 512],
                        start=True,
                        stop=True,
                    )
                    if diag:
                        nc.vector.tensor_add(st[:, 0:128], st[:, 0:128], mask_sb)
                    state[i] = st

                def process(i):
                    h, kb, o, w, diag = tiles[i]
                    st = state.pop(i)
                    pt = ppt.tile([128, 512], bf16, tag="pt", name="pt")
                    nc.scalar.activation(
                        pt[:, 0:w], st[:, 0:w], Exp, scale=SCALE
                    )
                    if kb == 0:
                        acc_t = pacc.tile(
                            [128, 512], f32r, tag="acc", name="acc"
                        )
                        pv_t = psPV.tile([128, 512], f32, tag="pv", name="pv")
                        grp["cur"] = (acc_t, pv_t)
                    acc, pv = grp["cur"]
                    if kb == 0:
                        nc.vector.tensor_copy(acc, pt)
                    else:
                        nc.vector.tensor_add(
                            acc[:, o:512], acc[:, o:512], pt[:, 0:w]
                        )
                    nc.tensor.matmul(
                        pv[:, o:512],
                        lhsT=vt[:, kb, h * 128 : (h + 1) * 128],
                        rhs=pt[:, 0:w],
                        start=(kb == 0),
                        stop=(kb == nk - 1),
                    )

                def group_end(h):
                    def go():
                        acc, pv = grp["cur"]
                        dn = psX.tile([128, 512], f32, tag="aux", name="dn")
                        nc.tensor.matmul(
                            dn, lhsT=ones_sb, rhs=acc, start=True, stop=True
                        )
                        rb = prb.tile([128, 512], f32, tag="rb", name="rb")
                        nc.vector.reciprocal_approx_fast(out=rb, in_=dn)
                        qsl = slice(qb * 512, (qb + 1) * 512)
                        nc.vector.tensor_mul(o2[h][:, qsl], pv, rb)
                    return go

                steps = [lambda: emit_st(0)]
                for i in range(1, len(tiles)):
                    def s(i=i):
                        emit_st(i)
                        process(i - 1)
                    steps.append(s)
                    if tiles[i - 1][1] == nk - 1:
                        steps.append(group_end(tiles[i - 1][0]))
                last = len(tiles) - 1
                steps.append(lambda: process(last))
                steps.append(group_end(tiles[last][0]))
                return steps

            # ---- out-projection unit closures for one q-block ----
            def p3_units(qb):
                units = []
                for ts2 in range(4):
                    for cb in range(4):
                        def go(ts2=ts2, cb=cb):
                            t0 = qb * 512 + ts2 * 128
                            trow = slice(t0, t0 + 128)
                            pos = psX.tile(
                                [128, 512], f32, tag="aux", name="pos"
                            )
                            for hd in range(HPC):
                                nc.tensor.matmul(
                                    pos,
                                    lhsT=o2[hd][:, trow],
                                    rhs=wps[hd][:, cb * 512 : (cb + 1) * 512],
                                    start=(hd == 0),
                                    stop=(hd == HPC - 1),
                                )
                            ob = pob.tile([128, 512], f32, tag="ob", name="ob")
                            if (ts2 + cb) % 2 == 0:
                                nc.vector.tensor_copy(ob, pos)
                            else:
                                nc.scalar.copy(ob, pos)
                            nc.sync.dma_start(
                                out=out[trow, cb * 512 : (cb + 1) * 512],
                                in_=ob,
                            )
                        units.append(go)
                return units

            def interleave(primary, fillers):
                seq = []
                fi = 0
                n = len(primary)
                for j, p in enumerate(primary):
                    seq.append(p)
                    tgt = (j + 1) * len(fillers) // n
                    while fi < tgt:
                        seq.append(fillers[fi])
                        fi += 1
                return seq

            sched = []
            sched += [load_x(1)] + chains(0)
            sched += [load_x(2)] + interleave(chains(1), att_steps(0))
            sched += [load_x(3)] + interleave(
                chains(2), att_steps(1) + p3_units(0)
            )
            sched += interleave(chains(3), att_steps(2) + p3_units(1))
            sched += interleave(att_steps(3), p3_units(2))
            sched += p3_units(3)
            for step in sched:
                step()
    nc.compile()
    return nc


def _get_program():
    if "nc" not in _CACHE:
        _CACHE["nc"] = _build_program()
    return _CACHE["nc"]


def make_in_maps(x, cos, sin, W_qkv, W_proj):
    """Host-side sharding: per-core input dicts."""
    import ml_dtypes

    bf16 = ml_dtypes.bfloat16
    x = np.asarray(x, dtype=np.float32)
    cos = np.asarray(cos, dtype=np.float32)
    sin = np.asarray(sin, dtype=np.float32)
    W_qkv = np.asarray(W_qkv, dtype=np.float32)
    W_proj = np.asarray(W_proj, dtype=np.float32)

    cosT = np.ascontiguousarray(np.tile(cos.T, (2, 1)).astype(bf16))  # [128,T]
    sinT = np.ascontiguousarray(
        np.concatenate([-sin.T, sin.T], axis=0).astype(bf16)
    )
    k_idx = np.arange(128)[:, None]
    c_idx = np.arange(128)[None, :]
    trimask = np.where(k_idx <= c_idx, 0.0, -1.0e30).astype(np.float32)
    onesr = np.ones((128, 128), dtype=np.float32)

    in_maps = []
    for core in range(NCORES):
        b, hg = core // 4, core % 4
        csl = slice(hg * 512, (hg + 1) * 512)
        wqk_np = np.concatenate(
            [W_qkv[:, csl], W_qkv[:, C + hg * 512 : C + (hg + 1) * 512]],
            axis=1,
        )  # [C, 1024]
        # lhsT blocks per m-tile, contiguous: [8, 128, KT*128]
        wqkg_np = np.ascontiguousarray(
            wqk_np.reshape(KT, 128, 8, 128)
            .transpose(2, 1, 0, 3)
            .reshape(8, 128, KT * 128)
            .astype(bf16)
        )
        wv_np = np.ascontiguousarray(
            W_qkv[:, 2 * C + hg * 512 : 2 * C + (hg + 1) * 512]
            .reshape(KT, 128, 512)
            .transpose(1, 0, 2)
            .reshape(128, KT * 512)
            .astype(bf16)
        )
        wp_np = np.ascontiguousarray(
            W_proj[hg * 512 : (hg + 1) * 512, :].astype(bf16)
        )
        xT_np = np.ascontiguousarray(x[b].T.astype(bf16))
        in_maps.append(
            {
                "xT": xT_np,
                "wqkg": wqkg_np,
                "wv": wv_np,
                "wp": wp_np,
                "onesr": onesr,
                "cosT": cosT,
                "sinTs": sinT,
                "trimask": trimask,
            }
        )
    return in_maps


def kernel(x, cos, sin, W_qkv, W_proj):
    from concourse.bass_utils import run_bass_kernel_spmd

    nc = _get_program()
    in_maps = make_in_maps(x, cos, sin, W_qkv, W_proj)
    trace = bool(int(os.environ.get("KERNEL_TRACE", "0")))
    res = run_bass_kernel_spmd(
        nc, in_maps, core_ids=list(range(NCORES)), trace=trace
    )
    if trace:
        _CACHE["last_results"] = res
        if res.exec_time_ns is not None:
            print(f"HW exec time: {res.exec_time_ns} ns")

    out = np.zeros((B, T, C), dtype=np.float32)
    for core in range(NCORES):
        out[core // 4] += res.results[core]["out"]
    return out


# revision 20
# speedup vs baseline: 1.5217x; 1.0362x over previous
"""Causal self-attention (B=2, T=2048, C=2048, H=16, D=128) on 8 trn2 cores.

Sharding: tensor-parallel over heads x data-parallel over batch.
Core c handles batch c//4, heads [4*(c%4) .. 4*(c%4)+4). Each core computes
qkv projection for its 4 heads, RoPE, causal attention, and a partial
output projection (its heads' rows of W_proj); the host sums the 4 partials
per batch.

All matmul operands are bf16 (PSUM accumulation stays fp32): bf16 enables
the fast-weight-load path so LDWEIGHTS fully overlaps the previous matmul
(fp32 HIGH mode serializes a 128-cycle LDW per matmul). Q^T/K^T/V stay
resident in SBUF (no DRAM scratch).

The whole kernel is one software-pipelined stream, interleaved so the
ACT (exp) and DVE (RoPE/mask/denominator-accumulate/evacuations) work of
attention hides under PE-dense projection chains:

  seg 0: qkv chains t-block 0
  seg 1: qkv chains tb1  + attention q-block 0 tiles as fillers
  seg 2: qkv chains tb2  + attention qb1 + out-proj rows of qb0
  seg 3: qkv chains tb3  + attention qb2 + out-proj qb1
  seg 4: attention qb3 with out-proj qb2 as filler
  seg 5: out-proj qb3

Attention (S^T orientation, one tile in flight ahead of the exp):
  S^T[k,q] = K^T.T @ Q^T   (diagonal tiles trimmed to exact causal width;
  the strictly-upper 128x128 triangle gets an additive -1e30 mask on DVE)
  P^T = exp(S^T / sqrt(D)) on ACT, bf16 (no max subtraction: scores O(5))
  denominator: P^T tiles accumulated into an SBUF f32 tile on DVE; one
  ones^T @ acc matmul per (qb, head) replicates per-q sums across
  partitions (saves the per-tile ones-matmul's 33% extra PE streaming)
  O^T[d,q] += V.T @ P^T in PSUM, normalized by reciprocal(denom) on DVE.
Out-projection (out[t,c] = sum_h O_h^T.T @ Wp_h) consumes O^T directly;
evacuations alternate DVE/ACT to balance engine load.
"""

import math
import os

import numpy as np

B, T, C = 2, 2048, 2048
H, D = 16, 128
HPC = 4  # heads per core
NCORES = 8
KT = C // 128  # 16 contraction tiles
NTB = T // 512  # 4 t-blocks

_CACHE = {}


def _build_program():
    import concourse.tile as tile
    from concourse import bacc, mybir

    f32 = mybir.dt.float32
    f32r = mybir.dt.float32r
    bf16 = mybir.dt.bfloat16
    Exp = mybir.ActivationFunctionType.Exp
    SCALE = 1.0 / math.sqrt(float(D))

    nc = bacc.Bacc(
        "TRN2", target_bir_lowering=False, debug=False, num_devices=NCORES
    )

    # x pre-packed per t-block: xg[tb, p, k*512+c] = x[tb*512+c, k*128+p],
    # so each t-block (and sub-range) is one linear DMA.
    xg = nc.dram_tensor(
        "xg", [NTB, 128, KT * 512], bf16, kind="ExternalInput"
    ).ap()
    wqkg = nc.dram_tensor(
        "wqkg", [8, 128, KT * 128], bf16, kind="ExternalInput"
    ).ap()
    wv = nc.dram_tensor("wv", [128, KT * 512], bf16, kind="ExternalInput").ap()
    wp = nc.dram_tensor("wp", [HPC * 128, C], bf16, kind="ExternalInput").ap()
    onesr = nc.dram_tensor("onesr", [128, 128], f32r, kind="ExternalInput").ap()
    onesb = nc.dram_tensor("onesb", [128, 128], bf16, kind="ExternalInput").ap()
    cosT = nc.dram_tensor("cosT", [128, T], bf16, kind="ExternalInput").ap()
    sinTs = nc.dram_tensor("sinTs", [128, T], bf16, kind="ExternalInput").ap()
    trimask = nc.dram_tensor(
        "trimask", [128, 128], f32, kind="ExternalInput"
    ).ap()
    out = nc.dram_tensor("out", [T, C], f32, kind="ExternalOutput").ap()

    with tile.TileContext(nc) as tc:
        with (
            tc.tile_pool(name="consts", bufs=1) as consts,
            tc.tile_pool(name="pers", bufs=1) as pers,
            tc.tile_pool(name="ppt", bufs=3) as ppt,
            tc.tile_pool(name="pacc", bufs=2) as pacc,
            tc.tile_pool(name="prb", bufs=2) as prb,
            tc.tile_pool(name="pob", bufs=4) as pob,
            tc.tile_pool(name="p1w", bufs=1) as p1w,
            tc.tile_pool(name="p1x", bufs=2) as p1x,
            tc.tile_pool(name="p1e", bufs=3) as p1e,
            tc.tile_pool(name="psA", bufs=2, space="PSUM") as psA,
            tc.tile_pool(name="psST", bufs=2, space="PSUM") as psST,
            tc.tile_pool(name="psPV", bufs=2, space="PSUM") as psPV,
            tc.tile_pool(name="psDN", bufs=1, space="PSUM") as psDN,
            tc.tile_pool(name="psPOS", bufs=1, space="PSUM") as psPOS,
        ):
            # ---- persistent SBUF tensors ----
            qts = [
                pers.tile([128, T], bf16, tag=f"qt{h}", name=f"qt{h}")
                for h in range(HPC)
            ]
            kts = [
                pers.tile([128, T], bf16, tag=f"kt{h}", name=f"kt{h}")
                for h in range(HPC)
            ]
            vt = pers.tile([128, KT, 512], bf16, tag="vt", name="vt")
            o2 = [
                pers.tile([128, T], bf16, tag=f"o2{h}", name=f"o2{h}")
                for h in range(HPC)
            ]
            wps = [
                pers.tile([128, T], bf16, tag=f"wp{i}", name=f"wp{i}")
                for i in range(HPC)
            ]

            # ---- input DMAs: weights + first x block first, the rest
            # behind them; big/cold loads on the scalar queue. ----
            wqkg_sb = p1w.tile([128, 8, KT * 128], bf16, tag="wqkg")
            wv_sb = p1w.tile([128, KT, 512], bf16, tag="wv")
            cos_sb = p1w.tile([128, T], bf16, tag="cos")
            sin_sb = p1w.tile([128, T], bf16, tag="sin")
            ones_sb = consts.tile([128, 128], f32r, tag="ones")
            onesb_sb = consts.tile([128, 128], bf16, tag="onesb")
            mask_sb = consts.tile([128, 128], f32, tag="mask")

            MORD = (0, 4, 1, 5, 2, 6, 3, 7)
            xtbs = [None] * NTB

            def load_x(tb, queues=(None,)):
                def go():
                    xtb = p1x.tile(
                        [128, KT, 512], bf16, tag="xtb", name=f"xtb{tb}"
                    )
                    xtbs[tb] = xtb
                    ng = len(queues)
                    kg = KT // ng
                    for g, q in enumerate(queues):
                        (q or nc.sync).dma_start(
                            out=xtb[:, g * kg : (g + 1) * kg],
                            in_=xg[tb][:, g * kg * 512 : (g + 1) * kg * 512],
                        )
                return go

            def load_wm(m, q=None):
                (q or nc.sync).dma_start(out=wqkg_sb[:, m, :], in_=wqkg[m])

            # Startup loads fanned across issue queues: the first chain
            # needs wm0 + the first x quarter as early as possible.
            load_wm(0)
            load_wm(4, nc.gpsimd)
            load_x(0, (nc.sync, nc.sync, nc.gpsimd, nc.gpsimd))()
            for m in (1, 5, 2, 6, 3, 7):
                load_wm(m, nc.gpsimd)
            nc.scalar.dma_start(out=cos_sb, in_=cosT)
            nc.scalar.dma_start(out=sin_sb, in_=sinTs)
            nc.scalar.dma_start(out=ones_sb, in_=onesr)
            nc.scalar.dma_start(out=onesb_sb, in_=onesb)
            nc.scalar.dma_start(out=mask_sb, in_=trimask)
            nc.scalar.dma_start(
                out=wv_sb, in_=wv.rearrange("p (k c) -> p k c", k=KT)
            )
            for i in range(HPC):
                nc.scalar.dma_start(
                    out=wps[i], in_=wp[i * 128 : (i + 1) * 128, :]
                )

            # ---- phase-1 chain closures ----
            def chain_qk(tb, m):
                def go():
                    tsl = slice(tb * 512, (tb + 1) * 512)
                    ps = psA.tile([128, 512], f32, tag="chain", name="psqk")
                    for k in range(KT):
                        nc.tensor.matmul(
                            ps,
                            lhsT=wqkg_sb[:, m, k * 128 : (k + 1) * 128],
                            rhs=xtbs[tb][:, k, :],
                            start=(k == 0),
                            stop=(k == KT - 1),
                        )
                    # RoPE fused with PSUM evacuation, bf16 out.
                    dst = (qts[m] if m < 4 else kts[m - 4])[:, tsl]
                    tmp = p1e.tile([128, 512], f32, tag="rtmp", name="rtmp")
                    nc.vector.tensor_mul(
                        tmp[0:64], ps[64:128], sin_sb[0:64, tsl]
                    )
                    nc.vector.tensor_mul(
                        tmp[64:128], ps[0:64], sin_sb[64:128, tsl]
                    )
                    nc.vector.tensor_mul(dst, ps, cos_sb[:, tsl])
                    nc.vector.tensor_add(dst, dst, tmp)
                return go

            def chain_v(tb, tsub):
                def go():
                    csl = slice(tsub * 128, (tsub + 1) * 128)
                    psv = psA.tile([128, 512], f32, tag="chain", name="psv")
                    for k in range(KT):
                        nc.tensor.matmul(
                            psv,
                            lhsT=xtbs[tb][:, k, csl],
                            rhs=wv_sb[:, k],
                            start=(k == 0),
                            stop=(k == KT - 1),
                        )
                    nc.scalar.copy(vt[:, tb * 4 + tsub, :], psv)
                return go

            def chains(tb):
                cs = [chain_qk(tb, m) for m in MORD]
                cs += [chain_v(tb, tsub) for tsub in range(4)]
                return cs

            # ---- attention step closures for one q-block ----
            # dn_tiles: accumulate the softmax denominator with per-tile
            # ones^T @ P^T matmuls on the PE instead of DVE adds into an
            # SBUF tile — used for qb3, whose attention runs without a
            # projection-chain segment to hide DVE work under.
            def att_steps(qb, dn_tiles=False):
                nk = 4 * (qb + 1)
                tiles = []
                for h in range(HPC):
                    for kb in range(nk):
                        j = kb - qb * 4
                        o = j * 128 if j >= 0 else 0
                        tiles.append((h, kb, o, 512 - o, j >= 0))
                state = {}
                grp = {}

                def emit_st(i):
                    h, kb, o, w, diag = tiles[i]
                    st = psST.tile([128, 512], f32, tag="st", name="st")
                    nc.tensor.matmul(
                        st[:, 0:w],
                        lhsT=kts[h][:, kb * 128 : (kb + 1) * 128],
                        rhs=qts[h][:, qb * 512 + o : (qb + 1) * 512],
                        start=True,
                        stop=True,
                    )
                    if diag:
                        nc.vector.tensor_add(st[:, 0:128], st[:, 0:128], mask_sb)
                    state[i] = st

                def process(i):
                    h, kb, o, w, diag = tiles[i]
                    st = state.pop(i)
                    pt = ppt.tile([128, 512], bf16, tag="pt", name="pt")
                    nc.scalar.activation(
                        pt[:, 0:w], st[:, 0:w], Exp, scale=SCALE
                    )
                    if kb == 0:
                        acc_t = None
                        if not dn_tiles:
                            acc_t = pacc.tile(
                                [128, 512], f32r, tag="acc", name="acc"
                            )
                        else:
                            grp["dn"] = psDN.tile(
                                [128, 512], f32, tag="dnt", name="dnt"
                            )
                        pv_t = psPV.tile([128, 512], f32, tag="pv", name="pv")
                        grp["cur"] = (acc_t, pv_t)
                    acc, pv = grp["cur"]
                    if dn_tiles:
                        nc.tensor.matmul(
                            grp["dn"][:, o:512],
                            lhsT=onesb_sb,
                            rhs=pt[:, 0:w],
                            start=(kb == 0),
                            stop=(kb == nk - 1),
                        )
                    elif kb == 0:
                        nc.vector.tensor_copy(acc, pt)
                    else:
                        nc.vector.tensor_add(
                            acc[:, o:512], acc[:, o:512], pt[:, 0:w]
                        )
                    nc.tensor.matmul(
                        pv[:, o:512],
                        lhsT=vt[:, kb, h * 128 : (h + 1) * 128],
                        rhs=pt[:, 0:w],
                        start=(kb == 0),
                        stop=(kb == nk - 1),
                    )

                def group_end(h):
                    def go():
                        acc, pv = grp["cur"]
                        if dn_tiles:
                            dn = grp["dn"]
                        else:
                            dn = psDN.tile(
                                [128, 512], f32, tag="dnt", name="dn"
                            )
                            nc.tensor.matmul(
                                dn, lhsT=ones_sb, rhs=acc, start=True,
                                stop=True,
                            )
                        rb = prb.tile([128, 512], f32, tag="rb", name="rb")
                        nc.vector.reciprocal_approx_fast(out=rb, in_=dn)
                        qsl = slice(qb * 512, (qb + 1) * 512)
                        nc.vector.tensor_mul(o2[h][:, qsl], pv, rb)
                    return go

                steps = [lambda: emit_st(0)]
                for i in range(1, len(tiles)):
                    def s(i=i):
                        emit_st(i)
                        process(i - 1)
                    steps.append(s)
                    if tiles[i - 1][1] == nk - 1:
                        steps.append(group_end(tiles[i - 1][0]))
                last = len(tiles) - 1
                steps.append(lambda: process(last))
                steps.append(group_end(tiles[last][0]))
                return steps

            # ---- out-projection unit closures for one q-block ----
            def p3_units(qb):
                units = []
                for ts2 in range(4):
                    for cb in range(4):
                        def go(ts2=ts2, cb=cb):
                            t0 = qb * 512 + ts2 * 128
                            trow = slice(t0, t0 + 128)
                            # qb3's units run after all attention: alternate
                            # the two single-buf pools so evacuation of unit
                            # i overlaps the matmuls of unit i+1.
                            pool = (
                                psDN
                                if qb == 3 and (ts2 * 4 + cb) % 2
                                else psPOS
                            )
                            pos = pool.tile(
                                [128, 512], f32, tag="dnt" if pool is psDN
                                else "pos", name="pos"
                            )
                            for hd in range(HPC):
                                nc.tensor.matmul(
                                    pos,
                                    lhsT=o2[hd][:, trow],
                                    rhs=wps[hd][:, cb * 512 : (cb + 1) * 512],
                                    start=(hd == 0),
                                    stop=(hd == HPC - 1),
                                )
                            ob = pob.tile([128, 512], f32, tag="ob", name="ob")
                            if (ts2 + cb) % 2 == 0:
                                nc.vector.tensor_copy(ob, pos)
                            else:
                                nc.scalar.copy(ob, pos)
                            nc.sync.dma_start(
                                out=out[trow, cb * 512 : (cb + 1) * 512],
                                in_=ob,
                            )
                        units.append(go)
                return units

            def interleave(primary, fillers):
                seq = []
                fi = 0
                n = len(primary)
                for j, p in enumerate(primary):
                    seq.append(p)
                    tgt = (j + 1) * len(fillers) // n
                    while fi < tgt:
                        seq.append(fillers[fi])
                        fi += 1
                return seq

            sched = []
            sched += [load_x(1)] + chains(0)
            sched += [load_x(2)] + interleave(chains(1), att_steps(0))
            sched += [load_x(3)] + interleave(
                chains(2), att_steps(1) + p3_units(0)
            )
            sched += interleave(chains(3), att_steps(2) + p3_units(1))
            sched += interleave(att_steps(3, dn_tiles=True), p3_units(2))
            sched += p3_units(3)
            for step in sched:
                step()
    nc.compile()
    return nc


def _get_program():
    if "nc" not in _CACHE:
        _CACHE["nc"] = _build_program()
    return _CACHE["nc"]


def make_in_maps(x, cos, sin, W_qkv, W_proj):
    """Host-side sharding: per-core input dicts."""
    import ml_dtypes

    bf16 = ml_dtypes.bfloat16
    x = np.asarray(x, dtype=np.float32)
    cos = np.asarray(cos, dtype=np.float32)
    sin = np.asarray(sin, dtype=np.float32)
    W_qkv = np.asarray(W_qkv, dtype=np.float32)
    W_proj = np.asarray(W_proj, dtype=np.float32)

    cosT = np.ascontiguousarray(np.tile(cos.T, (2, 1)).astype(bf16))  # [128,T]
    sinT = np.ascontiguousarray(
        np.concatenate([-sin.T, sin.T], axis=0).astype(bf16)
    )
    k_idx = np.arange(128)[:, None]
    c_idx = np.arange(128)[None, :]
    trimask = np.where(k_idx <= c_idx, 0.0, -1.0e30).astype(np.float32)
    onesr = np.ones((128, 128), dtype=np.float32)
    onesb_np = np.ones((128, 128), dtype=bf16)

    in_maps = []
    for core in range(NCORES):
        b, hg = core // 4, core % 4
        csl = slice(hg * 512, (hg + 1) * 512)
        wqk_np = np.concatenate(
            [W_qkv[:, csl], W_qkv[:, C + hg * 512 : C + (hg + 1) * 512]],
            axis=1,
        )  # [C, 1024]
        # lhsT blocks per m-tile, contiguous: [8, 128, KT*128]
        wqkg_np = np.ascontiguousarray(
            wqk_np.reshape(KT, 128, 8, 128)
            .transpose(2, 1, 0, 3)
            .reshape(8, 128, KT * 128)
            .astype(bf16)
        )
        wv_np = np.ascontiguousarray(
            W_qkv[:, 2 * C + hg * 512 : 2 * C + (hg + 1) * 512]
            .reshape(KT, 128, 512)
            .transpose(1, 0, 2)
            .reshape(128, KT * 512)
            .astype(bf16)
        )
        wp_np = np.ascontiguousarray(
            W_proj[hg * 512 : (hg + 1) * 512, :].astype(bf16)
        )
        xg_np = np.ascontiguousarray(
            x[b]
            .T.reshape(KT, 128, NTB, 512)
            .transpose(2, 1, 0, 3)
            .reshape(NTB, 128, KT * 512)
            .astype(bf16)
        )
        in_maps.append(
            {
                "xg": xg_np,
                "wqkg": wqkg_np,
                "wv": wv_np,
                "wp": wp_np,
                "onesr": onesr,
                "onesb": onesb_np,
                "cosT": cosT,
                "sinTs": sinT,
                "trimask": trimask,
            }
        )
    return in_maps


def kernel(x, cos, sin, W_qkv, W_proj):
    from concourse.bass_utils import run_bass_kernel_spmd

    nc = _get_program()
    in_maps = make_in_maps(x, cos, sin, W_qkv, W_proj)
    trace = bool(int(os.environ.get("KERNEL_TRACE", "0")))
    res = run_bass_kernel_spmd(
        nc, in_maps, core_ids=list(range(NCORES)), trace=trace
    )
    if trace:
        _CACHE["last_results"] = res
        if res.exec_time_ns is not None:
            print(f"HW exec time: {res.exec_time_ns} ns")

    out = np.zeros((B, T, C), dtype=np.float32)
    for core in range(NCORES):
        out[core // 4] += res.results[core]["out"]
    return out
